# revision 10
# baseline (speedup 1.0000x reference)
"""NNUE network forward pass on 8 Trainium2 NeuronCores (Bass/Tile).

Math (per reference):
    white_ft = clip(white @ ft_w.T + ft_b, 0, 1)        # [B, 512]
    black_ft = clip(black @ ft_w.T + ft_b, 0, 1)        # [B, 512]
    x = relu(concat([white_ft, black_ft], 1) @ fc1_w.T + fc1_b)  # [B, 256]
    out = x @ fc2_w.T + fc2_b                           # [B]

Distribution: data-parallel over the batch — each of the 8 cores handles
B/8 = 512 rows end to end; weights are replicated.  No collectives.

Device layout trick: the feature-transform matmul is computed with the
weights stationary and the batch on the moving free dim, so its output
lands in PSUM as [h1=partitions, batch=free] — which is exactly the
transposed ("contraction on partitions") layout fc1 needs.  The only
transposes anywhere are the DMA-xbar transposed loads of the feature
tiles, fused into the HBM->SBUF DMAs.

Matmul precision: bf16 inputs (host-cast), fp32 PSUM accumulation.
"""

import sys

for _p in ("/opt/trn_rl_repo", "/opt/pypackages"):
    if _p not in sys.path:
        sys.path.append(_p)

import numpy as np
import ml_dtypes

import concourse.bass as bass
import concourse.mybir as mybir
import concourse.tile as tile
from concourse.bass_utils import run_bass_kernel_spmd
from concourse.vector_clock import ScopedClock

# ---------------------------------------------------------------------------
# Workaround for the pinned walrus rejecting the TileContext exit Drain when
# it carries more than one semaphore wait ("Too many sync wait commands"):
# keep one wait on the drain and move the rest onto single-wait nops that
# still precede the all-engine barrier.
# ---------------------------------------------------------------------------
_MAX_DRAIN_WAITS = 1


def _split_drain_and_barrier(self, tick_clock, wait_clock):
    nc = self.nc
    drain_inst = nc.sync.drain()
    wait_clock.add_sem_waits(
        drain_inst.ins, ScopedClock({None: tick_clock.global_clock})
    )
    si = drain_inst.ins.sync_info
    if si is not None and si.on_wait and len(si.on_wait) > _MAX_DRAIN_WAITS:
        waits = list(si.on_wait)
        drain_inst.ins.sync_info = mybir.SyncInfo(
            on_wait=waits[:_MAX_DRAIN_WAITS], on_update=list(si.on_update)
        )
        for w in waits[_MAX_DRAIN_WAITS:]:
            ni = nc.sync.nop(nofuse=True, hint="drain_wait_split")
            nsi = ni.ins.sync_info
            upd = list(nsi.on_update) if nsi is not None else []
            ni.ins.sync_info = mybir.SyncInfo(on_wait=[w], on_update=upd)

    nc.all_engine_barrier()
    assert self.sems is not None
    popped = nc._tile_sem_poison_stack.pop()
    assert popped is self._sem_poison
    nc.clear_and_free_semaphores(list(self.sems.allocated().values()))
    nc.all_engine_barrier()


tile.TileContext._drain_and_barrier = _split_drain_and_barrier


def _split_multi_waits(nc, max_waits=1):
    """Walrus in this env rejects instructions with more than one sync wait.
    Split extras onto same-engine NOPs inserted immediately before (engine
    program order makes the stall equivalent)."""
    n_split = 0
    for f in nc.m.functions:
        for blk in f.blocks:
            out = []
            for ins in blk.instructions:
                si = ins.sync_info
                if si is not None and si.on_wait and len(si.on_wait) > max_waits:
                    waits = list(si.on_wait)
                    for w in waits[max_waits:]:
                        nop = mybir.InstNoOp(
                            name=f"I-{nc.next_id()}", ins=[], outs=[])
                        nop.engine = ins.engine
                        nop.sync_info = mybir.SyncInfo(
                            on_wait=[w], on_update=[])
                        out.append(nop)
                        n_split += 1
                    ins.sync_info = mybir.SyncInfo(
                        on_wait=waits[:max_waits],
                        on_update=list(si.on_update))
                out.append(ins)
            blk.instructions[:] = out
    return n_split

# ---------------------------------------------------------------------------
# Problem shapes (hardcoded per the harness contract).
# ---------------------------------------------------------------------------
BATCH = 4096
K = 40960          # feature size
H1 = 512           # feature-transform width
H2 = 256           # fc1 width
N_CORES = 8
BC = BATCH // N_CORES   # batch rows per core = 512

BF16 = mybir.dt.bfloat16
F32 = mybir.dt.float32
AF = mybir.ActivationFunctionType

P = 128            # partitions


def build_bass(k_total=K, k_chunk=1024, fused_xpose=True, n_devices=N_CORES, feat_bufs=4):
    """Build the per-core Bass module.  k_total may be scaled down for tests."""
    assert k_total % k_chunk == 0 and k_chunk % P == 0
    n_chunks = k_total // k_chunk
    n_sub = k_chunk // P          # 128-row k-subtiles per chunk
    n_h = H1 // P                 # 4  h1 tiles
    n_j = 2 * H1 // P             # 8  fc1 contraction tiles
    n_h2 = H2 // P                # 2  fc1 output tiles
    n_b = BC // P                 # 4  batch subtiles (for fc2)

    nc = bass.Bass("TRN2", target_bir_lowering=False, debug=False,
                   num_devices=n_devices)

    wf = nc.dram_tensor("wf", [BC, k_total], BF16, kind="ExternalInput")
    bl = nc.dram_tensor("bl", [BC, k_total], BF16, kind="ExternalInput")
    ft_wT = nc.dram_tensor("ft_wT", [k_total, H1], BF16, kind="ExternalInput")
    fc1_wT = nc.dram_tensor("fc1_wT", [2 * H1, H2], BF16, kind="ExternalInput")
    fc2_w = nc.dram_tensor("fc2_w", [H2, 1], BF16, kind="ExternalInput")
    ft_b = nc.dram_tensor("ft_b", [P, n_h], F32, kind="ExternalInput")
    fc1_b = nc.dram_tensor("fc1_b", [P, n_h2], F32, kind="ExternalInput")
    fc2_b = nc.dram_tensor("fc2_b", [P, 1], F32, kind="ExternalInput")
    out = nc.dram_tensor("out", [BC, 1], F32, kind="ExternalOutput")

    with tile.TileContext(nc) as tc:
        with (
            tc.tile_pool(name="consts", bufs=1) as consts,
            tc.tile_pool(name="feats", bufs=feat_bufs) as feats,
            tc.tile_pool(name="wpool", bufs=feat_bufs) as wpool,
            tc.tile_pool(name="ftout", bufs=1) as ftout,
            tc.tile_pool(name="small", bufs=1) as small,
        ):
            # --- constants / small weights -------------------------------
            ft_b_sb = consts.tile([P, n_h], F32, tag="ft_b")
            nc.gpsimd.dma_start(ft_b_sb[:], ft_b[:])
            fc1_b_sb = consts.tile([P, n_h2], F32, tag="fc1_b")
            nc.gpsimd.dma_start(fc1_b_sb[:], fc1_b[:])
            fc2_b_sb = consts.tile([P, 1], F32, tag="fc2_b")
            nc.gpsimd.dma_start(fc2_b_sb[:], fc2_b[:])
            fc1w_sb = consts.tile([P, n_j, H2], BF16, tag="fc1w")
            nc.gpsimd.dma_start(
                fc1w_sb[:], fc1_wT.rearrange("(c p) n -> p c n", p=P)
            )
            w2_sb = consts.tile([P, n_h2], BF16, tag="w2")
            nc.gpsimd.dma_start(
                w2_sb[:], fc2_w.rearrange("(c p) o -> p (c o)", p=P)
            )

            # --- stage A: feature transform ------------------------------
            # 8 PSUM accumulation chains: (perspective, h1-tile), each
            # [128 h1, 512 batch] fp32, accumulated over all of K.
            psA_cm = tc.tile_pool(name="psA", bufs=1, space="PSUM")
            psA = psA_cm.__enter__()
            pa = [
                psA.tile([P, BC], F32, tag=f"psA_{pi}_{h}", name=f"psA_{pi}_{h}")
                for pi in range(2)
                for h in range(n_h)
            ]

            for ci in range(n_chunks):
                k0 = ci * k_chunk
                # transposed feature tiles: [128 k, n_sub, 512 b]
                xw = feats.tile([P, n_sub, BC], BF16, tag="xw")
                xb = feats.tile([P, n_sub, BC], BF16, tag="xb")
                if fused_xpose:
                    nc.sync.dma_start_transpose(
                        xw[:], wf[:, k0:k0 + k_chunk])
                    nc.sync.dma_start_transpose(
                        xb[:], bl[:, k0:k0 + k_chunk])
                else:
                    for c in range(n_sub):
                        s = slice(k0 + c * P, k0 + (c + 1) * P)
                        nc.sync.dma_start_transpose(xw[:, c, :], wf[:, s])
                        nc.sync.dma_start_transpose(xb[:, c, :], bl[:, s])
                # weight chunk: [128 k, n_sub, 512 h1]
                wt = wpool.tile([P, n_sub, H1], BF16, tag="wt")
                nc.gpsimd.dma_start(
                    wt[:],
                    ft_wT[k0:k0 + k_chunk, :].rearrange(
                        "(c p) n -> p c n", p=P),
                )
                first = ci == 0
                last = ci == n_chunks - 1
                for c in range(n_sub):
                    for h in range(n_h):
                        for pi, x in ((0, xw), (1, xb)):
                            nc.tensor.matmul(
                                pa[pi * n_h + h][:],
                                wt[:, c, h * P:(h + 1) * P],
                                x[:, c, :],
                                start=first and c == 0,
                                stop=last and c == n_sub - 1,
                            )

            # --- eviction: clip(x + b, 0, 1), cast bf16 ------------------
            # output tiles double as fc1's transposed input [j, b]
            ft_t = []
            for pi in range(2):
                for h in range(n_h):
                    t_relu = small.tile([P, BC], F32, tag=f"relu_{pi}_{h}", name=f"relu_{pi}_{h}")
                    nc.scalar.activation(
                        t_relu[:], pa[pi * n_h + h][:], AF.Relu,
                        bias=ft_b_sb[:, h:h + 1],
                    )
                    t = ftout.tile([P, BC], BF16, tag=f"ft_{pi}_{h}", name=f"ft_{pi}_{h}")
                    nc.vector.tensor_scalar_min(t[:], t_relu[:], 1.0)
                    ft_t.append(t)

            psA_cm.__exit__(None, None, None)
            psB_cm = tc.tile_pool(name="psB", bufs=1, space="PSUM")
            psB = psB_cm.__enter__()

            # --- fc1: x2[h2, b] = relu(fc1_wT.T @ combinedT + b) ---------
            x2 = []
            for h2t in range(n_h2):
                pb = psB.tile([P, BC], F32, tag=f"psB_{h2t}", name=f"psB_{h2t}")
                for j in range(n_j):
                    nc.tensor.matmul(
                        pb[:],
                        fc1w_sb[:, j, h2t * P:(h2t + 1) * P],
                        ft_t[j][:],
                        start=j == 0,
                        stop=j == n_j - 1,
                    )
                t2 = small.tile([P, BC], BF16, tag=f"x2_{h2t}", name=f"x2_{h2t}")
                nc.scalar.activation(
                    t2[:], pb[:], AF.Relu, bias=fc1_b_sb[:, h2t:h2t + 1]
                )
                x2.append(t2)

            # --- fc2: out[b] = x2[:, b] . w2 + b2 ------------------------
            for bt in range(n_b):
                pc = psB.tile([P, 1], F32, tag=f"psC_{bt}", name=f"psC_{bt}")
                for h2t in range(n_h2):
                    nc.tensor.matmul(
                        pc[:],
                        x2[h2t][:, bt * P:(bt + 1) * P],
                        w2_sb[:, h2t:h2t + 1],
                        start=h2t == 0,
                        stop=h2t == n_h2 - 1,
                    )
                o_sb = small.tile([P, 1], F32, tag=f"o_{bt}", name=f"o_{bt}")
                nc.scalar.activation(
                    o_sb[:], pc[:], AF.Identity, bias=fc2_b_sb[:]
                )
                nc.gpsimd.dma_start(out[bt * P:(bt + 1) * P, :], o_sb[:])

            psB_cm.__exit__(None, None, None)

    _split_multi_waits(nc)
    return nc


# ---------------------------------------------------------------------------
# Host side
# ---------------------------------------------------------------------------
def _to_bf16(a):
    """Fast fp32 -> bf16 with round-to-nearest-even, via bit ops."""
    u = a.view(np.uint32)
    rounded = u + 0x7FFF + ((u >> 16) & 1)
    return (rounded >> 16).astype(np.uint16).view(ml_dtypes.bfloat16)


_NC_CACHE = {}


def _get_nc():
    if "nc" not in _NC_CACHE:
        _NC_CACHE["nc"] = build_bass()
    return _NC_CACHE["nc"]


def kernel(white_features, black_features, ft_w, ft_b, fc1_w, fc1_b,
           fc2_w, fc2_b, **kwargs):
    nc = _get_nc()

    wf16 = _to_bf16(np.ascontiguousarray(white_features, np.float32))
    bl16 = _to_bf16(np.ascontiguousarray(black_features, np.float32))
    ft_wT = _to_bf16(np.ascontiguousarray(ft_w.T, np.float32))
    fc1_wT = _to_bf16(np.ascontiguousarray(fc1_w.T, np.float32))
    fc2_wc = _to_bf16(np.ascontiguousarray(
        fc2_w.reshape(H2, 1), np.float32))
    ft_b_c = np.ascontiguousarray(
        np.asarray(ft_b, np.float32).reshape(H1 // P, P).T)
    fc1_b_c = np.ascontiguousarray(
        np.asarray(fc1_b, np.float32).reshape(H2 // P, P).T)
    fc2_b_c = np.full((P, 1), np.asarray(fc2_b, np.float32).reshape(()),
                      np.float32)

    in_maps = []
    for c in range(N_CORES):
        rows = slice(c * BC, (c + 1) * BC)
        in_maps.append({
            "wf": wf16[rows], "bl": bl16[rows],
            "ft_wT": ft_wT, "fc1_wT": fc1_wT, "fc2_w": fc2_wc,
            "ft_b": ft_b_c, "fc1_b": fc1_b_c, "fc2_b": fc2_b_c,
        })

    res = run_bass_kernel_spmd(
        nc, in_maps, core_ids=list(range(N_CORES)),
        **kwargs,
    )
    full = np.concatenate(
        [res.results[c]["out"].reshape(BC) for c in range(N_CORES)])
    if kwargs:
        return full.astype(np.float32), res
    return full.astype(np.float32)


# revision 11
# speedup vs baseline: 1.1743x; 1.1743x over previous
"""NNUE network forward pass on 8 Trainium2 NeuronCores (Bass/Tile).

Math (per reference):
    white_ft = clip(white @ ft_w.T + ft_b, 0, 1)        # [B, 512]
    black_ft = clip(black @ ft_w.T + ft_b, 0, 1)        # [B, 512]
    x = relu(concat([white_ft, black_ft], 1) @ fc1_w.T + fc1_b)  # [B, 256]
    out = x @ fc2_w.T + fc2_b                           # [B]

Distribution: data-parallel over the batch — each of the 8 cores handles
B/8 = 512 rows end to end; weights are replicated.  No collectives.

Device layout trick: the feature-transform matmul is computed with the
weights stationary and the batch on the moving free dim, so its output
lands in PSUM as [h1=partitions, batch=free] — which is exactly the
transposed ("contraction on partitions") layout fc1 needs.  The only
transposes anywhere are the DMA-xbar transposed loads of the feature
tiles, fused into the HBM->SBUF DMAs.

Matmul precision: bf16 inputs (host-cast), fp32 PSUM accumulation.
"""

import sys

for _p in ("/opt/trn_rl_repo", "/opt/pypackages"):
    if _p not in sys.path:
        sys.path.append(_p)

import numpy as np
import ml_dtypes

import concourse.bass as bass
import concourse.mybir as mybir
import concourse.tile as tile
from concourse.bass_utils import run_bass_kernel_spmd
from concourse.vector_clock import ScopedClock

# ---------------------------------------------------------------------------
# Workaround for the pinned walrus rejecting the TileContext exit Drain when
# it carries more than one semaphore wait ("Too many sync wait commands"):
# keep one wait on the drain and move the rest onto single-wait nops that
# still precede the all-engine barrier.
# ---------------------------------------------------------------------------
_MAX_DRAIN_WAITS = 1


def _split_drain_and_barrier(self, tick_clock, wait_clock):
    nc = self.nc
    drain_inst = nc.sync.drain()
    wait_clock.add_sem_waits(
        drain_inst.ins, ScopedClock({None: tick_clock.global_clock})
    )
    si = drain_inst.ins.sync_info
    if si is not None and si.on_wait and len(si.on_wait) > _MAX_DRAIN_WAITS:
        waits = list(si.on_wait)
        drain_inst.ins.sync_info = mybir.SyncInfo(
            on_wait=waits[:_MAX_DRAIN_WAITS], on_update=list(si.on_update)
        )
        for w in waits[_MAX_DRAIN_WAITS:]:
            ni = nc.sync.nop(nofuse=True, hint="drain_wait_split")
            nsi = ni.ins.sync_info
            upd = list(nsi.on_update) if nsi is not None else []
            ni.ins.sync_info = mybir.SyncInfo(on_wait=[w], on_update=upd)

    nc.all_engine_barrier()
    assert self.sems is not None
    popped = nc._tile_sem_poison_stack.pop()
    assert popped is self._sem_poison
    nc.clear_and_free_semaphores(list(self.sems.allocated().values()))
    nc.all_engine_barrier()


tile.TileContext._drain_and_barrier = _split_drain_and_barrier


def _split_multi_waits(nc, max_waits=1):
    """Walrus in this env rejects instructions with more than one sync wait.
    Split extras onto same-engine NOPs inserted immediately before (engine
    program order makes the stall equivalent)."""
    n_split = 0
    for f in nc.m.functions:
        for blk in f.blocks:
            out = []
            for ins in blk.instructions:
                si = ins.sync_info
                if si is not None and si.on_wait and len(si.on_wait) > max_waits:
                    waits = list(si.on_wait)
                    for w in waits[max_waits:]:
                        nop = mybir.InstNoOp(
                            name=f"I-{nc.next_id()}", ins=[], outs=[])
                        nop.engine = ins.engine
                        nop.sync_info = mybir.SyncInfo(
                            on_wait=[w], on_update=[])
                        out.append(nop)
                        n_split += 1
                    ins.sync_info = mybir.SyncInfo(
                        on_wait=waits[:max_waits],
                        on_update=list(si.on_update))
                out.append(ins)
            blk.instructions[:] = out
    return n_split

# ---------------------------------------------------------------------------
# Problem shapes (hardcoded per the harness contract).
# ---------------------------------------------------------------------------
BATCH = 4096
K = 40960          # feature size
H1 = 512           # feature-transform width
H2 = 256           # fc1 width
N_CORES = 8
BC = BATCH // N_CORES   # batch rows per core = 512

BF16 = mybir.dt.bfloat16
F32 = mybir.dt.float32
AF = mybir.ActivationFunctionType

P = 128            # partitions


def build_bass(k_total=K, k_chunk=1024, fused_xpose=True, n_devices=N_CORES, feat_bufs=4):
    """Build the per-core Bass module.  k_total may be scaled down for tests."""
    assert k_total % k_chunk == 0 and k_chunk % P == 0
    n_chunks = k_total // k_chunk
    n_sub = k_chunk // P          # 128-row k-subtiles per chunk
    n_h = H1 // P                 # 4  h1 tiles
    n_j = 2 * H1 // P             # 8  fc1 contraction tiles
    n_h2 = H2 // P                # 2  fc1 output tiles
    n_b = BC // P                 # 4  batch subtiles (for fc2)

    nc = bass.Bass("TRN2", target_bir_lowering=False, debug=False,
                   num_devices=n_devices)

    wf = nc.dram_tensor("wf", [BC, k_total], BF16, kind="ExternalInput")
    bl = nc.dram_tensor("bl", [BC, k_total], BF16, kind="ExternalInput")
    ft_wn = nc.dram_tensor("ft_wn", [H1, k_total], BF16, kind="ExternalInput")
    fc1_wT = nc.dram_tensor("fc1_wT", [2 * H1, H2], BF16, kind="ExternalInput")
    fc2_w = nc.dram_tensor("fc2_w", [H2, 1], BF16, kind="ExternalInput")
    ft_b = nc.dram_tensor("ft_b", [P, n_h], F32, kind="ExternalInput")
    fc1_b = nc.dram_tensor("fc1_b", [P, n_h2], F32, kind="ExternalInput")
    fc2_b = nc.dram_tensor("fc2_b", [P, 1], F32, kind="ExternalInput")
    out = nc.dram_tensor("out", [BC, 1], F32, kind="ExternalOutput")

    with tile.TileContext(nc) as tc:
        with (
            tc.tile_pool(name="consts", bufs=1) as consts,
            tc.tile_pool(name="feats", bufs=feat_bufs) as feats,
            tc.tile_pool(name="wpool", bufs=feat_bufs) as wpool,
            tc.tile_pool(name="ftout", bufs=1) as ftout,
            tc.tile_pool(name="small", bufs=1) as small,
        ):
            # --- constants / small weights -------------------------------
            ft_b_sb = consts.tile([P, n_h], F32, tag="ft_b")
            nc.gpsimd.dma_start(ft_b_sb[:], ft_b[:])
            fc1_b_sb = consts.tile([P, n_h2], F32, tag="fc1_b")
            nc.gpsimd.dma_start(fc1_b_sb[:], fc1_b[:])
            fc2_b_sb = consts.tile([P, 1], F32, tag="fc2_b")
            nc.gpsimd.dma_start(fc2_b_sb[:], fc2_b[:])
            fc1w_sb = consts.tile([P, n_j, H2], BF16, tag="fc1w")
            nc.gpsimd.dma_start(
                fc1w_sb[:], fc1_wT.rearrange("(c p) n -> p c n", p=P)
            )
            w2_sb = consts.tile([P, n_h2], BF16, tag="w2")
            nc.gpsimd.dma_start(
                w2_sb[:], fc2_w.rearrange("(c p) o -> p (c o)", p=P)
            )

            # --- stage A: feature transform ------------------------------
            # 8 PSUM accumulation chains: (perspective, h1-tile), each
            # [128 h1, 512 batch] fp32, accumulated over all of K.
            psA_cm = tc.tile_pool(name="psA", bufs=1, space="PSUM")
            psA = psA_cm.__enter__()
            pa = [
                psA.tile([P, BC], F32, tag=f"psA_{pi}_{h}", name=f"psA_{pi}_{h}")
                for pi in range(2)
                for h in range(n_h)
            ]

            for ci in range(n_chunks):
                k0 = ci * k_chunk
                # transposed feature tiles: [128 k, n_sub, 512 b]
                xw = feats.tile([P, n_sub, BC], BF16, tag="xw")
                xb = feats.tile([P, n_sub, BC], BF16, tag="xb")
                if fused_xpose:
                    nc.sync.dma_start_transpose(
                        xw[:], wf[:, k0:k0 + k_chunk])
                    nc.sync.dma_start_transpose(
                        xb[:], bl[:, k0:k0 + k_chunk])
                else:
                    for c in range(n_sub):
                        s = slice(k0 + c * P, k0 + (c + 1) * P)
                        nc.sync.dma_start_transpose(xw[:, c, :], wf[:, s])
                        nc.sync.dma_start_transpose(xb[:, c, :], bl[:, s])
                # weight chunk: [128 k, n_sub, 512 h1]
                wt = wpool.tile([P, n_sub, H1], BF16, tag="wt")
                nc.sync.dma_start_transpose(
                    wt[:], ft_wn[:, k0:k0 + k_chunk])
                first = ci == 0
                last = ci == n_chunks - 1
                for c in range(n_sub):
                    for h in range(n_h):
                        for pi, x in ((0, xw), (1, xb)):
                            nc.tensor.matmul(
                                pa[pi * n_h + h][:],
                                wt[:, c, h * P:(h + 1) * P],
                                x[:, c, :],
                                start=first and c == 0,
                                stop=last and c == n_sub - 1,
                            )

            # --- eviction: clip(x + b, 0, 1), cast bf16 ------------------
            # output tiles double as fc1's transposed input [j, b]
            ft_t = []
            for pi in range(2):
                for h in range(n_h):
                    t_relu = small.tile([P, BC], F32, tag=f"relu_{pi}_{h}", name=f"relu_{pi}_{h}")
                    nc.scalar.activation(
                        t_relu[:], pa[pi * n_h + h][:], AF.Relu,
                        bias=ft_b_sb[:, h:h + 1],
                    )
                    t = ftout.tile([P, BC], BF16, tag=f"ft_{pi}_{h}", name=f"ft_{pi}_{h}")
                    nc.vector.tensor_scalar_min(t[:], t_relu[:], 1.0)
                    ft_t.append(t)

            psA_cm.__exit__(None, None, None)
            psB_cm = tc.tile_pool(name="psB", bufs=1, space="PSUM")
            psB = psB_cm.__enter__()

            # --- fc1: x2[h2, b] = relu(fc1_wT.T @ combinedT + b) ---------
            x2 = []
            for h2t in range(n_h2):
                pb = psB.tile([P, BC], F32, tag=f"psB_{h2t}", name=f"psB_{h2t}")
                for j in range(n_j):
                    nc.tensor.matmul(
                        pb[:],
                        fc1w_sb[:, j, h2t * P:(h2t + 1) * P],
                        ft_t[j][:],
                        start=j == 0,
                        stop=j == n_j - 1,
                    )
                t2 = small.tile([P, BC], BF16, tag=f"x2_{h2t}", name=f"x2_{h2t}")
                nc.scalar.activation(
                    t2[:], pb[:], AF.Relu, bias=fc1_b_sb[:, h2t:h2t + 1]
                )
                x2.append(t2)

            # --- fc2: out[b] = x2[:, b] . w2 + b2 ------------------------
            for bt in range(n_b):
                pc = psB.tile([P, 1], F32, tag=f"psC_{bt}", name=f"psC_{bt}")
                for h2t in range(n_h2):
                    nc.tensor.matmul(
                        pc[:],
                        x2[h2t][:, bt * P:(bt + 1) * P],
                        w2_sb[:, h2t:h2t + 1],
                        start=h2t == 0,
                        stop=h2t == n_h2 - 1,
                    )
                o_sb = small.tile([P, 1], F32, tag=f"o_{bt}", name=f"o_{bt}")
                nc.scalar.activation(
                    o_sb[:], pc[:], AF.Identity, bias=fc2_b_sb[:]
                )
                nc.gpsimd.dma_start(out[bt * P:(bt + 1) * P, :], o_sb[:])

            psB_cm.__exit__(None, None, None)

    _split_multi_waits(nc)
    return nc


# ---------------------------------------------------------------------------
# Host side
# ---------------------------------------------------------------------------
def _to_bf16(a):
    """Fast fp32 -> bf16 with round-to-nearest-even, via bit ops."""
    u = a.view(np.uint32)
    rounded = u + 0x7FFF + ((u >> 16) & 1)
    return (rounded >> 16).astype(np.uint16).view(ml_dtypes.bfloat16)


_NC_CACHE = {}


def _get_nc():
    if "nc" not in _NC_CACHE:
        _NC_CACHE["nc"] = build_bass()
    return _NC_CACHE["nc"]


def kernel(white_features, black_features, ft_w, ft_b, fc1_w, fc1_b,
           fc2_w, fc2_b, **kwargs):
    nc = _get_nc()

    wf16 = _to_bf16(np.ascontiguousarray(white_features, np.float32))
    bl16 = _to_bf16(np.ascontiguousarray(black_features, np.float32))
    ft_wn = _to_bf16(np.ascontiguousarray(ft_w, np.float32))
    fc1_wT = _to_bf16(np.ascontiguousarray(fc1_w.T, np.float32))
    fc2_wc = _to_bf16(np.ascontiguousarray(
        fc2_w.reshape(H2, 1), np.float32))
    ft_b_c = np.ascontiguousarray(
        np.asarray(ft_b, np.float32).reshape(H1 // P, P).T)
    fc1_b_c = np.ascontiguousarray(
        np.asarray(fc1_b, np.float32).reshape(H2 // P, P).T)
    fc2_b_c = np.full((P, 1), np.asarray(fc2_b, np.float32).reshape(()),
                      np.float32)

    in_maps = []
    for c in range(N_CORES):
        rows = slice(c * BC, (c + 1) * BC)
        in_maps.append({
            "wf": wf16[rows], "bl": bl16[rows],
            "ft_wn": ft_wn, "fc1_wT": fc1_wT, "fc2_w": fc2_wc,
            "ft_b": ft_b_c, "fc1_b": fc1_b_c, "fc2_b": fc2_b_c,
        })

    res = run_bass_kernel_spmd(
        nc, in_maps, core_ids=list(range(N_CORES)),
        **kwargs,
    )
    full = np.concatenate(
        [res.results[c]["out"].reshape(BC) for c in range(N_CORES)])
    if kwargs:
        return full.astype(np.float32), res
    return full.astype(np.float32)


# revision 12
# speedup vs baseline: 1.2005x; 1.0223x over previous
"""NNUE network forward pass on 8 Trainium2 NeuronCores (Bass/Tile).

Math (per reference):
    white_ft = clip(white @ ft_w.T + ft_b, 0, 1)        # [B, 512]
    black_ft = clip(black @ ft_w.T + ft_b, 0, 1)        # [B, 512]
    x = relu(concat([white_ft, black_ft], 1) @ fc1_w.T + fc1_b)  # [B, 256]
    out = x @ fc2_w.T + fc2_b                           # [B]

Distribution: data-parallel over the batch — each of the 8 cores handles
B/8 = 512 rows end to end; weights are replicated.  No collectives.

Device layout trick: the feature-transform matmul is computed with the
weights stationary and the batch on the moving free dim, so its output
lands in PSUM as [h1=partitions, batch=free] — which is exactly the
transposed ("contraction on partitions") layout fc1 needs.  The only
transposes anywhere are the DMA-xbar transposed loads of the feature
tiles, fused into the HBM->SBUF DMAs.

Matmul precision: bf16 inputs (host-cast), fp32 PSUM accumulation.
"""

import sys

for _p in ("/opt/trn_rl_repo", "/opt/pypackages"):
    if _p not in sys.path:
        sys.path.append(_p)

import numpy as np
import ml_dtypes

import concourse.bass as bass
import concourse.mybir as mybir
import concourse.tile as tile
from concourse.bass_utils import run_bass_kernel_spmd
from concourse.vector_clock import ScopedClock

# ---------------------------------------------------------------------------
# Workaround for the pinned walrus rejecting the TileContext exit Drain when
# it carries more than one semaphore wait ("Too many sync wait commands"):
# keep one wait on the drain and move the rest onto single-wait nops that
# still precede the all-engine barrier.
# ---------------------------------------------------------------------------
_MAX_DRAIN_WAITS = 1


def _split_drain_and_barrier(self, tick_clock, wait_clock):
    nc = self.nc
    drain_inst = nc.sync.drain()
    wait_clock.add_sem_waits(
        drain_inst.ins, ScopedClock({None: tick_clock.global_clock})
    )
    si = drain_inst.ins.sync_info
    if si is not None and si.on_wait and len(si.on_wait) > _MAX_DRAIN_WAITS:
        waits = list(si.on_wait)
        drain_inst.ins.sync_info = mybir.SyncInfo(
            on_wait=waits[:_MAX_DRAIN_WAITS], on_update=list(si.on_update)
        )
        for w in waits[_MAX_DRAIN_WAITS:]:
            ni = nc.sync.nop(nofuse=True, hint="drain_wait_split")
            nsi = ni.ins.sync_info
            upd = list(nsi.on_update) if nsi is not None else []
            ni.ins.sync_info = mybir.SyncInfo(on_wait=[w], on_update=upd)

    nc.all_engine_barrier()
    assert self.sems is not None
    popped = nc._tile_sem_poison_stack.pop()
    assert popped is self._sem_poison
    nc.clear_and_free_semaphores(list(self.sems.allocated().values()))
    nc.all_engine_barrier()


tile.TileContext._drain_and_barrier = _split_drain_and_barrier


def _split_multi_waits(nc, max_waits=1):
    """Walrus in this env rejects instructions with more than one sync wait.
    Split extras onto same-engine NOPs inserted immediately before (engine
    program order makes the stall equivalent)."""
    n_split = 0
    for f in nc.m.functions:
        for blk in f.blocks:
            out = []
            for ins in blk.instructions:
                si = ins.sync_info
                if si is not None and si.on_wait and len(si.on_wait) > max_waits:
                    waits = list(si.on_wait)
                    for w in waits[max_waits:]:
                        nop = mybir.InstNoOp(
                            name=f"I-{nc.next_id()}", ins=[], outs=[])
                        nop.engine = ins.engine
                        nop.sync_info = mybir.SyncInfo(
                            on_wait=[w], on_update=[])
                        out.append(nop)
                        n_split += 1
                    ins.sync_info = mybir.SyncInfo(
                        on_wait=waits[:max_waits],
                        on_update=list(si.on_update))
                out.append(ins)
            blk.instructions[:] = out
    return n_split

# ---------------------------------------------------------------------------
# Problem shapes (hardcoded per the harness contract).
# ---------------------------------------------------------------------------
BATCH = 4096
K = 40960          # feature size
H1 = 512           # feature-transform width
H2 = 256           # fc1 width
N_CORES = 8
BC = BATCH // N_CORES   # batch rows per core = 512

BF16 = mybir.dt.bfloat16
F32 = mybir.dt.float32
AF = mybir.ActivationFunctionType

P = 128            # partitions


def build_bass(k_total=K, k_chunk=2048, fused_xpose=True, n_devices=N_CORES, feat_bufs=3):
    """Build the per-core Bass module.  k_total may be scaled down for tests."""
    assert k_total % k_chunk == 0 and k_chunk % P == 0
    n_chunks = k_total // k_chunk
    n_sub = k_chunk // P          # 128-row k-subtiles per chunk
    n_h = H1 // P                 # 4  h1 tiles
    n_j = 2 * H1 // P             # 8  fc1 contraction tiles
    n_h2 = H2 // P                # 2  fc1 output tiles
    n_b = BC // P                 # 4  batch subtiles (for fc2)

    nc = bass.Bass("TRN2", target_bir_lowering=False, debug=False,
                   num_devices=n_devices)

    wf = nc.dram_tensor("wf", [BC, k_total], BF16, kind="ExternalInput")
    bl = nc.dram_tensor("bl", [BC, k_total], BF16, kind="ExternalInput")
    ft_wn = nc.dram_tensor("ft_wn", [H1, k_total], BF16, kind="ExternalInput")
    fc1_wT = nc.dram_tensor("fc1_wT", [2 * H1, H2], BF16, kind="ExternalInput")
    fc2_w = nc.dram_tensor("fc2_w", [H2, 1], BF16, kind="ExternalInput")
    ft_b = nc.dram_tensor("ft_b", [P, n_h], F32, kind="ExternalInput")
    fc1_b = nc.dram_tensor("fc1_b", [P, n_h2], F32, kind="ExternalInput")
    fc2_b = nc.dram_tensor("fc2_b", [P, 1], F32, kind="ExternalInput")
    out = nc.dram_tensor("out", [BC, 1], F32, kind="ExternalOutput")

    with tile.TileContext(nc) as tc:
        with (
            tc.tile_pool(name="consts", bufs=1) as consts,
            tc.tile_pool(name="feats", bufs=feat_bufs) as feats,
            tc.tile_pool(name="wpool", bufs=feat_bufs) as wpool,
            tc.tile_pool(name="ftout", bufs=1) as ftout,
            tc.tile_pool(name="small", bufs=1) as small,
        ):
            # --- constants / small weights -------------------------------
            ft_b_sb = consts.tile([P, n_h], F32, tag="ft_b")
            nc.gpsimd.dma_start(ft_b_sb[:], ft_b[:])
            fc1_b_sb = consts.tile([P, n_h2], F32, tag="fc1_b")
            nc.gpsimd.dma_start(fc1_b_sb[:], fc1_b[:])
            fc2_b_sb = consts.tile([P, 1], F32, tag="fc2_b")
            nc.gpsimd.dma_start(fc2_b_sb[:], fc2_b[:])
            fc1w_sb = consts.tile([P, n_j, H2], BF16, tag="fc1w")
            nc.gpsimd.dma_start(
                fc1w_sb[:], fc1_wT.rearrange("(c p) n -> p c n", p=P)
            )
            w2_sb = consts.tile([P, n_h2], BF16, tag="w2")
            nc.gpsimd.dma_start(
                w2_sb[:], fc2_w.rearrange("(c p) o -> p (c o)", p=P)
            )

            # --- stage A: feature transform ------------------------------
            # 8 PSUM accumulation chains: (perspective, h1-tile), each
            # [128 h1, 512 batch] fp32, accumulated over all of K.
            psA_cm = tc.tile_pool(name="psA", bufs=1, space="PSUM")
            psA = psA_cm.__enter__()
            pa = [
                psA.tile([P, BC], F32, tag=f"psA_{pi}_{h}", name=f"psA_{pi}_{h}")
                for pi in range(2)
                for h in range(n_h)
            ]

            for ci in range(n_chunks):
                k0 = ci * k_chunk
                # transposed feature tiles: [128 k, n_sub, 512 b]
                xw = feats.tile([P, n_sub, BC], BF16, tag="xw")
                xb = feats.tile([P, n_sub, BC], BF16, tag="xb")
                if fused_xpose:
                    nc.sync.dma_start_transpose(
                        xw[:], wf[:, k0:k0 + k_chunk])
                    nc.sync.dma_start_transpose(
                        xb[:], bl[:, k0:k0 + k_chunk])
                else:
                    for c in range(n_sub):
                        s = slice(k0 + c * P, k0 + (c + 1) * P)
                        nc.sync.dma_start_transpose(xw[:, c, :], wf[:, s])
                        nc.sync.dma_start_transpose(xb[:, c, :], bl[:, s])
                # weight chunk: [128 k, n_sub, 512 h1]
                wt = wpool.tile([P, n_sub, H1], BF16, tag="wt")
                nc.sync.dma_start_transpose(
                    wt[:], ft_wn[:, k0:k0 + k_chunk])
                first = ci == 0
                last = ci == n_chunks - 1
                for c in range(n_sub):
                    for h in range(n_h):
                        for pi, x in ((0, xw), (1, xb)):
                            nc.tensor.matmul(
                                pa[pi * n_h + h][:],
                                wt[:, c, h * P:(h + 1) * P],
                                x[:, c, :],
                                start=first and c == 0,
                                stop=last and c == n_sub - 1,
                            )

            # --- eviction: clip(x + b, 0, 1), cast bf16 ------------------
            # output tiles double as fc1's transposed input [j, b]
            ft_t = []
            for pi in range(2):
                for h in range(n_h):
                    t_relu = small.tile([P, BC], F32, tag=f"relu_{pi}_{h}", name=f"relu_{pi}_{h}")
                    nc.scalar.activation(
                        t_relu[:], pa[pi * n_h + h][:], AF.Relu,
                        bias=ft_b_sb[:, h:h + 1],
                    )
                    t = ftout.tile([P, BC], BF16, tag=f"ft_{pi}_{h}", name=f"ft_{pi}_{h}")
                    nc.vector.tensor_scalar_min(t[:], t_relu[:], 1.0)
                    ft_t.append(t)

            psA_cm.__exit__(None, None, None)
            psB_cm = tc.tile_pool(name="psB", bufs=1, space="PSUM")
            psB = psB_cm.__enter__()

            # --- fc1: x2[h2, b] = relu(fc1_wT.T @ combinedT + b) ---------
            x2 = []
            for h2t in range(n_h2):
                pb = psB.tile([P, BC], F32, tag=f"psB_{h2t}", name=f"psB_{h2t}")
                for j in range(n_j):
                    nc.tensor.matmul(
                        pb[:],
                        fc1w_sb[:, j, h2t * P:(h2t + 1) * P],
                        ft_t[j][:],
                        start=j == 0,
                        stop=j == n_j - 1,
                    )
                t2 = small.tile([P, BC], BF16, tag=f"x2_{h2t}", name=f"x2_{h2t}")
                nc.scalar.activation(
                    t2[:], pb[:], AF.Relu, bias=fc1_b_sb[:, h2t:h2t + 1]
                )
                x2.append(t2)

            # --- fc2: out[b] = x2[:, b] . w2 + b2 ------------------------
            for bt in range(n_b):
                pc = psB.tile([P, 1], F32, tag=f"psC_{bt}", name=f"psC_{bt}")
                for h2t in range(n_h2):
                    nc.tensor.matmul(
                        pc[:],
                        x2[h2t][:, bt * P:(bt + 1) * P],
                        w2_sb[:, h2t:h2t + 1],
                        start=h2t == 0,
                        stop=h2t == n_h2 - 1,
                    )
                o_sb = small.tile([P, 1], F32, tag=f"o_{bt}", name=f"o_{bt}")
                nc.scalar.activation(
                    o_sb[:], pc[:], AF.Identity, bias=fc2_b_sb[:]
                )
                nc.gpsimd.dma_start(out[bt * P:(bt + 1) * P, :], o_sb[:])

            psB_cm.__exit__(None, None, None)

    _split_multi_waits(nc)
    return nc


# ---------------------------------------------------------------------------
# Host side
# ---------------------------------------------------------------------------
def _to_bf16(a):
    """Fast fp32 -> bf16 with round-to-nearest-even, via bit ops."""
    u = a.view(np.uint32)
    rounded = u + 0x7FFF + ((u >> 16) & 1)
    return (rounded >> 16).astype(np.uint16).view(ml_dtypes.bfloat16)


_NC_CACHE = {}


def _get_nc():
    if "nc" not in _NC_CACHE:
        _NC_CACHE["nc"] = build_bass()
    return _NC_CACHE["nc"]


def kernel(white_features, black_features, ft_w, ft_b, fc1_w, fc1_b,
           fc2_w, fc2_b, **kwargs):
    nc = _get_nc()

    wf16 = _to_bf16(np.ascontiguousarray(white_features, np.float32))
    bl16 = _to_bf16(np.ascontiguousarray(black_features, np.float32))
    ft_wn = _to_bf16(np.ascontiguousarray(ft_w, np.float32))
    fc1_wT = _to_bf16(np.ascontiguousarray(fc1_w.T, np.float32))
    fc2_wc = _to_bf16(np.ascontiguousarray(
        fc2_w.reshape(H2, 1), np.float32))
    ft_b_c = np.ascontiguousarray(
        np.asarray(ft_b, np.float32).reshape(H1 // P, P).T)
    fc1_b_c = np.ascontiguousarray(
        np.asarray(fc1_b, np.float32).reshape(H2 // P, P).T)
    fc2_b_c = np.full((P, 1), np.asarray(fc2_b, np.float32).reshape(()),
                      np.float32)

    in_maps = []
    for c in range(N_CORES):
        rows = slice(c * BC, (c + 1) * BC)
        in_maps.append({
            "wf": wf16[rows], "bl": bl16[rows],
            "ft_wn": ft_wn, "fc1_wT": fc1_wT, "fc2_w": fc2_wc,
            "ft_b": ft_b_c, "fc1_b": fc1_b_c, "fc2_b": fc2_b_c,
        })

    res = run_bass_kernel_spmd(
        nc, in_maps, core_ids=list(range(N_CORES)),
        **kwargs,
    )
    full = np.concatenate(
        [res.results[c]["out"].reshape(BC) for c in range(N_CORES)])
    if kwargs:
        return full.astype(np.float32), res
    return full.astype(np.float32)
